# revision 23
# baseline (speedup 1.0000x reference)
"""Trainium2 Bass kernel for nn_ContrastiveLabeledLoss (segment_reduce).

loss = sum_c [ sum_{i in c} ||x_i - a_c||^2 ] / max(n_c - 1, 1),  a_c = x[first(c)]

Per-class expansion (w_c = [n_c>=2] / max(n_c-1,1)):

    D_c  = S2_c - 2 <S_c, a_c> + n_c ||a_c||^2,   loss = sum_c w_c D_c
    S_c  = sum_{i in c} x_i        (per-class vector sum)
    S2_c = sum_{i in c} ||x_i||^2  (per-class scalar)

Sharding: BY CLASS. Host stable-sorts samples by label (label-only metadata
preprocessing) and assigns 128 classes to each of the 8 cores (greedy balance),
so every class is fully local to one core: no collectives, no per-sample anchor
gather, no first-occurrence search on device (stable sort keeps the global
first occurrence as the first row of each segment).

Per core, per block of 128-sample tiles:
  DMA  streams the fp32 shard into the lower half of a [128, t, 512] bf16
       tile (SWDGE cast)
  DVE  squares x into the upper half (same-tensor mult -> 2x DVE mode) and
       builds the local one-hot [128, 128] per tile from host slot ids
  PE   one N=512 matmul per tile accumulates [S_c | per-class x^2 sums]
       into one PSUM bank
Epilogue: one 128-row indirect DMA fetches anchor rows; a few [128,256] DVE
ops and a ones-matmul fold everything into the scalar partial. Pad rows have
slot id 255 -> all-zero one-hot -> contribute nothing.
"""

import os
import sys

import numpy as np

sys.path.insert(0, "/opt/trn_rl_repo")

# Problem constants (hardcoded per harness contract).
N = 262144
D = 256
C = 1024
N_CORES = 8
CPC = C // N_CORES         # classes per core (= 128 = partition count)
P = 128
TPB = 30                   # 128-sample tiles per full block
BLOCKS = [30] * 8 + [18]   # 258 tiles = 33024 padded samples per core
T = sum(BLOCKS)
NS_PAD = T * P
PAD_SLOT = 255.0

_cached = {}


def _build_kernel():
    import concourse.bacc as bacc
    import concourse.bass as bass
    import concourse.mybir as mybir
    import concourse.tile as tile

    dt = mybir.dt
    Alu = mybir.AluOpType

    nc = bacc.Bacc(
        "TRN2",
        target_bir_lowering=False,
        debug=False,
        enable_asserts=False,
        num_devices=N_CORES,
    )

    x = nc.dram_tensor("x", [NS_PAD, D], dt.float32, kind="ExternalInput")
    losl = nc.dram_tensor("losl", [P, T], dt.bfloat16, kind="ExternalInput")
    iota = nc.dram_tensor("iota", [P, P], dt.bfloat16, kind="ExternalInput")
    aidx = nc.dram_tensor("aidx", [P, 1], dt.int32, kind="ExternalInput")
    wvec = nc.dram_tensor("wvec", [P, 1], dt.float32, kind="ExternalInput")
    nvec = nc.dram_tensor("nvec", [P, 1], dt.float32, kind="ExternalInput")
    ones = nc.dram_tensor("ones", [P, 1], dt.float32, kind="ExternalInput")
    part = nc.dram_tensor("part", [1, 1], dt.float32, kind="ExternalOutput")

    with tile.TileContext(nc) as tc:
        with (
            tc.tile_pool(name="singles", bufs=1) as singles,
            tc.tile_pool(name="xin", bufs=4) as xp,
            tc.tile_pool(name="oh", bufs=5) as ohp,
            tc.tile_pool(name="small", bufs=4) as smallp,
            tc.tile_pool(name="psum", bufs=1, space="PSUM") as psp,
        ):
            losl_sb = singles.tile([P, T], dt.bfloat16)
            nc.sync.dma_start(losl_sb[:], losl[:])
            iota_sb = singles.tile([P, P], dt.bfloat16)
            nc.sync.dma_start(iota_sb[:], iota[:])
            aidx_sb = singles.tile([P, 1], dt.int32)
            nc.sync.dma_start(aidx_sb[:], aidx[:])
            w_sb = singles.tile([P, 1], dt.float32)
            nc.sync.dma_start(w_sb[:], wvec[:])
            n_sb = singles.tile([P, 1], dt.float32)
            nc.sync.dma_start(n_sb[:], nvec[:])
            ones_sb = singles.tile([P, 1], dt.float32)
            nc.sync.dma_start(ones_sb[:], ones[:])

            # anchor rows: one row per partition = per local class slot
            crows = singles.tile([P, D], dt.float32)
            nc.gpsimd.indirect_dma_start(
                out=crows[:],
                out_offset=None,
                in_=x[:, :],
                in_offset=bass.IndirectOffsetOnAxis(ap=aidx_sb[:, 0:1], axis=0),
            )

            # materialized iota [P, TPB, P]: one-hot build then has a single
            # broadcast operand (any stride-0 operand forces 1x DVE mode, so
            # keep the other two APs unit-stride)
            iota_big = singles.tile([P, TPB, P], dt.bfloat16)
            nc.vector.tensor_copy(
                iota_big[:], iota_sb[:].unsqueeze(1).to_broadcast([P, TPB, P])
            )
            ps_S = psp.tile([P, 2 * D], dt.float32, tag="psS")

            t0 = 0
            for blk, tpb in enumerate(BLOCKS):
                xb = xp.tile([P, TPB, 2 * D], dt.bfloat16, tag="xb")
                nc.gpsimd.dma_start(
                    out=xb[:, 0:tpb, 0:D],
                    in_=x[t0 * P:(t0 + tpb) * P, :].rearrange(
                        "(b p) d -> p b d", p=P
                    ),
                )
                sl = slice(t0, t0 + tpb)
                # x^2 into the upper half: same-tensor unit-stride operands
                # keep DVE in its 2x mode
                nc.vector.tensor_tensor(
                    out=xb[:, 0:tpb, D:2 * D],
                    in0=xb[:, 0:tpb, 0:D],
                    in1=xb[:, 0:tpb, 0:D],
                    op=Alu.mult,
                )
                oh = ohp.tile([P, TPB, P], dt.bfloat16, tag="oh")
                nc.vector.tensor_tensor(
                    out=oh[:, 0:tpb, :],
                    in0=iota_big[:, 0:tpb, :],
                    in1=losl_sb[:, sl].unsqueeze(2).to_broadcast([P, tpb, P]),
                    op=Alu.is_equal,
                )
                for b in range(tpb):
                    t = t0 + b
                    nc.tensor.matmul(
                        out=ps_S[:, :],
                        lhsT=oh[:, b, :],
                        rhs=xb[:, b, :],
                        start=(t == 0),
                        stop=(t == T - 1),
                        skip_group_check=True,
                    )
                t0 += tpb

            # ---- epilogue: D_c = S2_c - 2<S_c,a_c> + n_c ||a_c||^2 ----
            s_sb = singles.tile([P, 2 * D], dt.float32)
            nc.vector.tensor_copy(s_sb[:], ps_S[:])
            sa = smallp.tile([P, D], dt.float32, tag="sa")
            nc.vector.tensor_mul(sa[:], s_sb[:, 0:D], crows[:])
            a2 = smallp.tile([P, D], dt.float32, tag="a2")
            nc.vector.tensor_mul(a2[:], crows[:], crows[:])
            sdota = smallp.tile([P, 1], dt.float32, tag="sdota")
            nc.vector.tensor_reduce(
                out=sdota[:], in_=sa[:], axis=mybir.AxisListType.X, op=Alu.add
            )
            a2r = smallp.tile([P, 1], dt.float32, tag="a2r")
            nc.vector.tensor_reduce(
                out=a2r[:], in_=a2[:], axis=mybir.AxisListType.X, op=Alu.add
            )
            s2 = smallp.tile([P, 1], dt.float32, tag="s2")
            nc.vector.tensor_reduce(
                out=s2[:], in_=s_sb[:, D:2 * D], axis=mybir.AxisListType.X,
                op=Alu.add,
            )
            na2 = smallp.tile([P, 1], dt.float32, tag="na2")
            nc.vector.tensor_mul(na2[:], a2r[:], n_sb[:])
            nc.vector.tensor_add(na2[:], na2[:], s2[:])
            # dcl = (s2 + n*||a||^2) - 2*<S,a>
            dcl = smallp.tile([P, 1], dt.float32, tag="dcl")
            nc.vector.scalar_tensor_tensor(
                dcl[:], sdota[:], -2.0, na2[:], op0=Alu.mult, op1=Alu.add
            )
            wd = smallp.tile([P, 1], dt.float32, tag="wd")
            nc.vector.tensor_mul(wd[:], dcl[:], w_sb[:])

            ps_fin = psp.tile([1, 1], dt.float32, tag="psfin")
            nc.tensor.matmul(
                out=ps_fin[:],
                lhsT=ones_sb[:],
                rhs=wd[:],
                start=True,
                stop=True,
                skip_group_check=True,
            )
            out_sb = smallp.tile([1, 1], dt.float32, tag="outsb")
            nc.vector.tensor_copy(out_sb[:], ps_fin[:])
            nc.sync.dma_start(part[:, :], out_sb[:])

    nc.compile()
    return nc


def _host_inputs(outputs: np.ndarray, labels: np.ndarray):
    """Class-sharded per-core in_maps (all label preprocessing host-side)."""
    import ml_dtypes

    lab = np.asarray(labels).astype(np.int64).ravel()
    counts = np.bincount(lab, minlength=C).astype(np.int64)
    perm = np.argsort(lab, kind="stable")
    seg = np.zeros(C + 1, dtype=np.int64)
    seg[1:] = np.cumsum(counts)

    # greedy balance: 128 classes per core, minimize max sample load
    order = np.argsort(-counts, kind="stable")
    load = np.zeros(N_CORES, dtype=np.int64)
    ncls = np.zeros(N_CORES, dtype=np.int64)
    core_classes = [[] for _ in range(N_CORES)]
    for c in order:
        best, bl = -1, None
        for r in range(N_CORES):
            if ncls[r] < CPC and (bl is None or load[r] < bl):
                best, bl = r, load[r]
        core_classes[best].append(int(c))
        load[best] += counts[c]
        ncls[best] += 1
    assert load.max() <= NS_PAD, f"core overflow: {load.max()} > {NS_PAD}"

    iota_t = np.tile(
        np.arange(P, dtype=np.float32), (P, 1)
    ).astype(ml_dtypes.bfloat16)
    ones = np.ones((P, 1), dtype=np.float32)

    in_maps = []
    for r in range(N_CORES):
        cls = core_classes[r]
        nr = int(load[r])
        rows = np.concatenate([perm[seg[c]:seg[c + 1]] for c in cls])
        slot_sizes = np.array([counts[c] for c in cls], dtype=np.int64)
        astart = np.zeros(P, dtype=np.int64)
        astart[1:] = np.cumsum(slot_sizes)[:-1]

        x_r = np.zeros((NS_PAD, D), dtype=np.float32)
        x_r[:nr] = outputs[rows]

        wcls = np.where(
            slot_sizes >= 2, 1.0 / np.maximum(slot_sizes - 1, 1), 0.0
        ).astype(np.float64)

        slot = np.full(NS_PAD, PAD_SLOT, dtype=np.float32)
        slot[:nr] = np.repeat(np.arange(P, dtype=np.float32), slot_sizes)
        losl = slot.reshape(T, P).T.astype(ml_dtypes.bfloat16)

        in_maps.append(
            {
                "x": x_r,
                "losl": np.ascontiguousarray(losl),
                "iota": iota_t,
                "aidx": astart.astype(np.int32).reshape(P, 1),
                "wvec": wcls.astype(np.float32).reshape(P, 1),
                "nvec": slot_sizes.astype(np.float32).reshape(P, 1),
                "ones": ones,
            }
        )
    return in_maps


def kernel(outputs, labels, num_classes):
    outputs = np.asarray(outputs, dtype=np.float32)
    labels = np.asarray(labels)
    assert outputs.shape == (N, D) and int(num_classes) == C

    if "nc" not in _cached:
        _cached["nc"] = _build_kernel()
    nc = _cached["nc"]

    from concourse.bass_utils import run_bass_kernel_spmd

    in_maps = _host_inputs(outputs, labels)
    res = run_bass_kernel_spmd(
        nc,
        in_maps,
        core_ids=list(range(N_CORES)),
        trace=bool(int(os.environ.get("KERNEL_TRACE", "0"))),
    )
    _cached["last_results"] = res
    total = np.float32(0.0)
    for r in range(N_CORES):
        total += res.results[r]["part"].reshape(-1)[0]
    return np.float32(total)
